# revision 25
# baseline (speedup 1.0000x reference)
"""Trainium2 Bass kernel for a 16-head attention block (d_model=1024, seq=4096).

Sharding: tensor-parallel over heads. Each of the 8 cores computes QKV
projections, RMSNorm(q,k), full softmax(QK^T)V attention for its 2 heads,
and a partial O-projection (its heads' slice of the contraction). The host
sums the 8 partial outputs (bf16 partials) and adds the output bias.

Per-core dataflow (k-first, attention is ACT/exp-bound so everything else
is arranged to hide under it):
  phase 1 (serial prefix, PE-bound): k,v projections only (fp32r, moving
           dim 256), RMSNorm(k) with wq*wk folded into the k side,
           PE-transpose k_hat into [64, s] fp32r tiles, V -> bf16 [k, 65]
           chunks with a fused ones column. The q side for the first
           q-tile is also produced here (bf16 GEMM, see below).
  phase 2 (ACT-bound steady state): per (q-tile 512, head):
           S[k,q] blocks via khatT.T @ qhatT (K=64 fp32r), exp on ACT ->
           bf16 probs, z[q,65] += probs_chunk.T @ V' with probs as the
           stationary operand (65-column moving operand halves PE time).
           Row 64 = softmax denominator; normalize in q-major on DVE,
           PE-transpose both heads at once into z_nT[d_local, s] bf16.
           The NEXT q-tile's q side runs under the exp shadow: bf16
           x @ Wq GEMM (N=128 bf16 runs at 1 cyc/row; fp32r would be 4x),
           RMSNorm(q) with a Newton-iteration rsqrt on DVE (keeps the
           ACT table on Exp), PE-transposes into qhatT.
  phase 3: out[s,dm] partial = z_nT.T @ WoT (bf16), PSUM->SBUF bf16 -> HBM,
           software-pipelined one q-tile behind attention.
"""

import numpy as np
from contextlib import ExitStack

import concourse.bass as bass
import concourse.tile as tile
from concourse import mybir
from concourse.masks import make_identity

F32 = mybir.dt.float32
F32R = mybir.dt.float32r
BF16 = mybir.dt.bfloat16
I32 = mybir.dt.int32
AF = mybir.ActivationFunctionType
ALU = mybir.AluOpType

D_MODEL = 1024
SEQ = 4096
N_HEADS = 16
D_HEAD = 64
N_CORES = 8
HEADS_LOCAL = 2
P = 128
DM_AUG = D_MODEL + P                     # 1152 rows: x^T plus ones-row block
NCH = DM_AUG // P                        # 9 contraction chunks
KV_LOCAL = 2 * HEADS_LOCAL * D_HEAD      # 256: [k0|k1|v0|v1]
Q_LOCAL = HEADS_LOCAL * D_HEAD           # 128: [q0|q1]
SB = SEQ // P                            # 32 s-blocks
QT = 8                                   # q-tiles of 512
QW = SEQ // QT                           # 512
QC = QW // P                             # 4 q-chunks of 128 per q-tile
KB = SEQ // P                            # 32 k-blocks
EXP_BATCH = 3
EPS = 1e-6
RSQRT_MAGIC = 0x5F3759DF


MAX_WAITS = 1


def _split_excess_waits(nc):
    """This walrus build rejects instructions carrying more than one or two
    sync-wait commands (CTRL and pseudo-DMA structs especially). Rewrite every
    instruction with more than MAX_WAITS waits into a chain of same-engine
    NoOps each carrying MAX_WAITS waits, followed by the original."""
    import bass_rust

    n_new = 0
    for f in nc.m.functions:
        for bb in f.blocks:
            changed = False
            out = []
            for ins in bb.instructions:
                si = ins.sync_info
                waits = list(si.on_wait) if si is not None and si.on_wait else []
                if len(waits) > MAX_WAITS:
                    changed = True
                    ncar = len(waits) - MAX_WAITS
                    for i in range(0, ncar, MAX_WAITS):
                        chunk = waits[i : min(i + MAX_WAITS, ncar)]
                        nop = mybir.InstNoOp(
                            name=f"{ins.name}-wsplit{i}", ins=[], outs=[]
                        )
                        nop.engine = ins.engine
                        nop.sync_info = bass_rust.SyncInfo(
                            on_wait=chunk, on_update=[]
                        )
                        out.append(nop)
                        n_new += 1
                    ins.sync_info = bass_rust.SyncInfo(
                        on_wait=waits[ncar:], on_update=si.on_update
                    )
                out.append(ins)
            if changed:
                bb.instructions = out
    return n_new


def build_core_kernel(split_waits=True):
    nc = bass.Bass()
    xtb = nc.declare_dram_parameter("xtb", [DM_AUG, SEQ], BF16, isOutput=False)
    wkvt = nc.declare_dram_parameter("wkvt", [DM_AUG, KV_LOCAL], BF16, isOutput=False)
    wqt = nc.declare_dram_parameter("wqt", [DM_AUG, Q_LOCAL], BF16, isOutput=False)
    wot = nc.declare_dram_parameter("wot", [P, D_MODEL], BF16, isOutput=False)
    wkc = nc.declare_dram_parameter("wkc", [D_HEAD, 1], F32, isOutput=False)
    out = nc.declare_dram_parameter("out", [SEQ, D_MODEL], BF16, isOutput=True)

    xtb_r = xtb.rearrange("(c p) s -> p c s", p=P)       # [128, 9, 4096]
    wkvt_r = wkvt.rearrange("(c p) f -> p c f", p=P)     # [128, 9, 256]
    wqt_r = wqt.rearrange("(c p) f -> p c f", p=P)       # [128, 9, 128]

    with ExitStack() as ctx:
        tc = ctx.enter_context(tile.TileContext(nc))

        const = ctx.enter_context(tc.tile_pool(name="const", bufs=1))
        persist = ctx.enter_context(tc.tile_pool(name="persist", bufs=1))

        # DMA order matters: the whole prefix waits on x chunk 0 + Wkv
        xb0 = const.tile([P, NCH, QW], BF16, name="xb0")
        nc.sync.dma_start(xb0[:, :, 0:QW // 2], xtb_r[:, :, 0 : QW // 2])
        wkv_sb = const.tile([P, NCH, KV_LOCAL], BF16)
        nc.sync.dma_start(wkv_sb[:], wkvt_r)
        nc.sync.dma_start(xb0[:, :, QW // 2 : QW], xtb_r[:, :, QW // 2 : QW])
        ident_f = const.tile([P, P], F32)
        make_identity(nc, ident_f)
        ident_r = const.tile([P, P], F32R)
        nc.scalar.activation(ident_r[:], ident_f[:], AF.Copy)
        ident_b = const.tile([P, P], BF16)
        nc.vector.tensor_copy(ident_b[:], ident_f[:])
        wkc_sb = const.tile([D_HEAD, 1], F32)
        nc.sync.dma_start(wkc_sb[:], wkc[:])
        eps_t = const.tile([P, 1], F32)
        nc.gpsimd.memset(eps_t[:], EPS)
        wq_sb = const.tile([P, NCH, Q_LOCAL], BF16)
        nc.sync.dma_start(wq_sb[:], wqt_r)
        wot_sb = const.tile([P, D_MODEL], BF16)
        nc.sync.dma_start(wot_sb[:], wot[:])

        # attention operands: q_hat/k_hat in [64, s] fp32r (K=64 contraction),
        # V' in [k, 65] bf16 per (head, k-block) with ones denominator column
        qhatT = [persist.tile([D_HEAD, SEQ], F32R, name=f"qhatT{h}") for h in range(2)]
        khatT = [persist.tile([D_HEAD, SEQ], F32R, name=f"khatT{h}") for h in range(2)]
        vp = persist.tile([P, HEADS_LOCAL, KB, D_HEAD + 1], BF16)
        nc.gpsimd.memset(vp[:, :, :, D_HEAD : D_HEAD + 1], 1.0)
        z_nT = persist.tile([P, SEQ], BF16)

        # resident bf16 copy of x^T (augmented): 8 chunk tiles of 512 s-cols,
        # loaded once (1KB descriptor runs; serves both kv- and q-GEMMs)
        xball = [xb0] + [
            persist.tile([P, NCH, QW], BF16, name=f"xb{d}") for d in range(1, QT)
        ]
        for d in range(1, QT):
            nc.sync.dma_start(xball[d][:], xtb_r[:, :, bass.ts(d, QW)])

        qnorm = ctx.enter_context(tc.tile_pool(name="qnorm", bufs=3))

        def emit_q_side_sb(sb, qpool, tpool, rsqrt_act=False):
            """bf16 x @ Wq for one s-block + RMSNorm(q) + PE transposes into
            qhatT. rsqrt_act: use ACT Rsqrt (phase-1 table) instead of the
            DVE Newton rsqrt (phase 2 keeps the ACT table on Exp)."""
            if True:
                ssl = bass.ts(sb, P)
                xbt = xball[sb // QC]
                xsl = bass.ts(sb % QC, P)
                qps = qpool.tile([P, Q_LOCAL], F32, name="qps", tag=qpool.name_tag)
                for c in range(NCH):
                    nc.tensor.matmul(
                        qps[:],
                        lhsT=xbt[:, c, xsl],
                        rhs=wq_sb[:, c, :],
                        start=(c == 0),
                        stop=(c == NCH - 1),
                    )
                qsb = qnorm.tile([P, Q_LOCAL], F32, name="qsb", tag="qsb")
                nc.vector.tensor_copy(qsb[:], qps[:])
                qg = qsb[:].rearrange("p (g d) -> p g d", g=2)
                sq = qnorm.tile([P, 2, D_HEAD], F32, name="qsq", tag="qsq")
                nc.vector.tensor_tensor(sq[:], qg, qg, ALU.mult)
                ss = qnorm.tile([P, 2], F32, name="qss", tag="qss")
                nc.vector.tensor_reduce(
                    ss[:], sq[:], axis=mybir.AxisListType.X, op=ALU.add
                )
                # rr = rsqrt(ss/64 + eps) via bit-trick seed + 2 Newton steps
                if rsqrt_act:
                    yrs = qnorm.tile([P, 2], F32, name="qrs", tag="qrs")
                    nc.scalar.activation(
                        yrs[:], ss[:], AF.Sqrt, bias=eps_t[:], scale=1.0 / D_HEAD
                    )
                    yact = qnorm.tile([P, 2], F32, name="qra", tag="qra")
                    nc.vector.reciprocal(yact[:], yrs[:])
                    y = yact[:]
                else:
                    ms = qnorm.tile([P, 2], F32, name="qms", tag="qms")
                    nc.vector.tensor_scalar(
                        ms[:], ss[:], 1.0 / D_HEAD, EPS, op0=ALU.mult, op1=ALU.add
                    )
                    xh = qnorm.tile([P, 2], F32, name="qxh", tag="qxh")
                    nc.vector.tensor_scalar(xh[:], ms[:], 0.5, None, op0=ALU.mult)
                    iy = qnorm.tile([P, 2], I32, name="qiy", tag="qiy")
                    nc.vector.tensor_scalar(
                        iy[:], ms[:].bitcast(I32), 1, None, op0=ALU.logical_shift_right
                    )
                    nc.vector.tensor_scalar(
                        iy[:], iy[:], -1, RSQRT_MAGIC, op0=ALU.mult, op1=ALU.add
                    )
                    y = iy[:].bitcast(F32)
                    for it in range(2):
                        y2 = qnorm.tile([P, 2], F32, name=f"qy2_{it}", tag=f"qy2_{it}")
                        nc.vector.tensor_tensor(y2[:], y, y, ALU.mult)
                        nc.vector.tensor_tensor(y2[:], y2[:], xh[:], ALU.mult)
                        nc.vector.tensor_scalar(
                            y2[:], y2[:], -1.0, 1.5, op0=ALU.mult, op1=ALU.add
                        )
                        yn = qnorm.tile([P, 2], F32, name=f"qyn_{it}", tag=f"qyn_{it}")
                        nc.vector.tensor_tensor(yn[:], y, y2[:], ALU.mult)
                        y = yn[:]
                q_hat = qnorm.tile([P, 2, D_HEAD], F32R, name="qhat", tag="qhat")
                nc.vector.tensor_tensor(
                    q_hat[:], qg, y[:, :, None].to_broadcast((P, 2, D_HEAD)), ALU.mult
                )
                for h in range(2):
                    pt = tpool.tile(
                        [D_HEAD, P], F32R, name="qpt", tag=tpool.name_tag
                    )
                    nc.tensor.transpose(pt[:], q_hat[:, h, :], ident_r[:])
                    nc.vector.tensor_copy(qhatT[h][:, ssl], pt[:])

        # ------------- phase 1: K/V projections + RMSNorm(k) + V' -------------
        with ExitStack() as p1:
            norm = p1.enter_context(tc.tile_pool(name="norm", bufs=8))
            qkps = p1.enter_context(tc.tile_pool(name="kvps", bufs=4, space="PSUM"))
            tps = p1.enter_context(tc.tile_pool(name="tps", bufs=4, space="PSUM"))

            class _P1Pool:
                name_tag = "kvps"

                @staticmethod
                def tile(shape, dt, name=None, tag=None):
                    return qkps.tile(shape, dt, name=name, tag="kvps")

            class _P1TPool:
                name_tag = "tps"

                @staticmethod
                def tile(shape, dt, name=None, tag=None):
                    return tps.tile(shape, dt, name=name, tag="tps")

            for sb in range(SB):
                ssl = bass.ts(sb, P)
                xbt = xball[sb // QC]
                xsl = bass.ts(sb % QC, P)

                kv_ps = qkps.tile([P, KV_LOCAL], F32, name="kv_ps", tag="kvps")
                for c in range(NCH):
                    nc.tensor.matmul(
                        kv_ps[:],
                        lhsT=xbt[:, c, xsl],
                        rhs=wkv_sb[:, c, :],
                        start=(c == 0),
                        stop=(c == NCH - 1),
                    )

                # RMSNorm stats for the 2 k heads
                k_ps = kv_ps[:, 0 : 2 * D_HEAD].rearrange("p (g d) -> p g d", g=2)
                sq = norm.tile([P, 2, D_HEAD], F32)
                nc.scalar.activation(sq[:], k_ps, AF.Square)
                ss = norm.tile([P, 2], F32)
                nc.vector.tensor_reduce(
                    ss[:], sq[:], axis=mybir.AxisListType.X, op=ALU.add
                )
                rs = norm.tile([P, 2], F32)
                nc.scalar.activation(
                    rs[:], ss[:], AF.Sqrt, bias=eps_t[:], scale=1.0 / D_HEAD
                )
                rr = norm.tile([P, 2], F32)
                nc.vector.reciprocal(rr[:], rs[:])

                k_hat = norm.tile([P, 2, D_HEAD], F32R)
                nc.vector.tensor_tensor(
                    k_hat[:],
                    k_ps,
                    rr[:, :, None].to_broadcast((P, 2, D_HEAD)),
                    ALU.mult,
                )

                # V chunks for both heads -> bf16 (ACT: same table as Square)
                nc.scalar.activation(
                    vp[:, :, sb, 0:D_HEAD],
                    kv_ps[:, 2 * D_HEAD : 4 * D_HEAD].rearrange(
                        "p (h d) -> p h d", h=2
                    ),
                    AF.Copy,
                )

                # transposes into [d, s]; wq*wk folded into the k side
                for h in range(2):
                    pt = tps.tile([D_HEAD, P], F32R, name="pt", tag="tps")
                    nc.tensor.transpose(pt[:], k_hat[:, h, :], ident_r[:])
                    if h == 0:
                        nc.scalar.activation(
                            khatT[h][:, ssl], pt[:], AF.Copy, scale=wkc_sb[:]
                        )
                    else:
                        nc.vector.tensor_scalar_mul(
                            khatT[h][:, ssl], pt[:], wkc_sb[:]
                        )

                # q side of the first q-tile, interleaved into the prefix
                # tail (ACT Sqrt: phase 1 owns the sqrt table)
                if sb >= 17 and (sb - 17) % 4 == 0:
                    emit_q_side_sb((sb - 17) // 4, _P1Pool, _P1TPool,
                                   rsqrt_act=True)



        # ---------- phase 2+3: attention with inlined O-projection ----------
        # PSUM banks: 2 score slots x3 banks, z accumulator 1 bank, shared
        # utility bank ("ops": O-proj / next-q-tile GEMM+transposes / ztp) = 8
        with ExitStack() as p2:
            spool = p2.enter_context(tc.tile_pool(name="sps", bufs=2, space="PSUM"))
            zqpool = p2.enter_context(tc.tile_pool(name="zqps", bufs=1, space="PSUM"))
            opool = p2.enter_context(tc.tile_pool(name="ops", bufs=1, space="PSUM"))
            ppool = p2.enter_context(tc.tile_pool(name="probs", bufs=4))
            znpool = p2.enter_context(tc.tile_pool(name="zn", bufs=3))
            rpool = p2.enter_context(tc.tile_pool(name="rcp", bufs=3))
            osb = p2.enter_context(tc.tile_pool(name="osb", bufs=4))

            class _P2Pool:
                name_tag = "ops"

                @staticmethod
                def tile(shape, dt, name=None, tag=None):
                    return opool.tile(shape, dt, name=name, tag="ops")

            def emit_oproj(qt, final=False):
                # final q-tile: S slots are free, so pipeline the matmuls
                # 2-wide through them and put half the copies on the idle ACT
                for sbl in range(QC):
                    sb = qt * QC + sbl
                    ot = osb.tile([P, D_MODEL], BF16, name="ot", tag="ot")
                    for half in range(2):
                        if final:
                            ops = spool.tile([P, QW], F32, name="ops", tag="sps")
                        else:
                            ops = opool.tile([P, QW], F32, name="ops", tag="ops")
                        nc.tensor.matmul(
                            ops[:],
                            lhsT=z_nT[:, bass.ts(sb, P)],
                            rhs=wot_sb[:, bass.ts(half, QW)],
                            start=True,
                            stop=True,
                        )
                        if final and half == 0:
                            nc.scalar.activation(
                                ot[:, bass.ts(half, QW)], ops[:], AF.Copy
                            )
                        else:
                            nc.vector.tensor_copy(ot[:, bass.ts(half, QW)], ops[:])
                    nc.sync.dma_start(out[bass.ts(sb, P), :], ot[:])

            for qt in range(QT):
                qsl = bass.ts(qt, QW)
                zn = znpool.tile([P, QC, P], BF16, name="zn", tag="zn")
                for h in range(HEADS_LOCAL):
                    zq = zqpool.tile([P, QC, D_HEAD + 1], F32, name="zq", tag="zq")
                    for kb0 in range(0, KB, EXP_BATCH):
                        nb = min(EXP_BATCH, KB - kb0)
                        sps = spool.tile(
                            [P, EXP_BATCH, QW], F32, name="sps", tag="sps"
                        )
                        for j in range(nb):
                            kb = kb0 + j
                            nc.tensor.matmul(
                                sps[:, j, :],
                                lhsT=khatT[h][:, bass.ts(kb, P)],
                                rhs=qhatT[h][:, qsl],
                                start=True,
                                stop=True,
                            )
                        probs = ppool.tile(
                            [P, EXP_BATCH, QW], BF16, name="probs", tag="probs"
                        )
                        nc.scalar.activation(
                            probs[:, 0:nb, :], sps[:, 0:nb, :], AF.Exp
                        )
                        # all 128 PV matmuls form ONE PSUM accumulation group
                        # (zq spans a single 2KB zero region): start marks the
                        # whole region pending-zero, each chunk's first touch
                        # overwrites, everything else accumulates
                        for j in range(nb):
                            kb = kb0 + j
                            for qc in range(QC):
                                nc.tensor.matmul(
                                    zq[:, qc, :],
                                    lhsT=probs[:, j, bass.ts(qc, P)],
                                    rhs=vp[:, h, kb, :],
                                    start=(kb == 0 and qc == 0),
                                    stop=(kb == KB - 1 and qc == QC - 1),
                                    skip_group_check=True,
                                )
                        # software-pipelined work emitted under the exp shadow:
                        # h0: O-projection of the previous q-tile
                        # h1: q side (GEMM+norm+transposes) of the next q-tile
                        if kb0 == 0 and h == 0 and qt > 0:
                            emit_oproj(qt - 1)
                        if kb0 == 0 and h == 1 and qt < QT - 1:
                            for sbl in range(QC):
                                emit_q_side_sb((qt + 1) * QC + sbl, _P2Pool, _P2Pool)
                    # normalize in q-major: z = z / rowsum (col 64)
                    rcp = rpool.tile([P, QC], F32, name="rcp", tag="rcp")
                    nc.vector.reciprocal(rcp[:], zq[:, :, D_HEAD])
                    nc.vector.tensor_tensor(
                        zn[:, :, bass.ts(h, D_HEAD)],
                        zq[:, :, 0:D_HEAD],
                        rcp[:, :, None].to_broadcast((P, QC, D_HEAD)),
                        ALU.mult,
                    )
                # transpose both heads at once into z_nT[d_local, s]
                ztp = opool.tile([P, QC, P], BF16, name="ztp", tag="ops")
                for qc in range(QC):
                    nc.tensor.transpose(ztp[:, qc, :], zn[:, qc, :], ident_b[:])
                    nc.vector.tensor_copy(
                        z_nT[:, qt * QW + qc * P : qt * QW + (qc + 1) * P],
                        ztp[:, qc, :],
                    )
            emit_oproj(QT - 1, final=True)

    if split_waits:
        _split_excess_waits(nc)
    return nc


def shard_inputs(x, Wqkv, bqkv, Wo, bo, wq, wk):
    import ml_dtypes

    x2 = np.ascontiguousarray(np.asarray(x, dtype=np.float32).reshape(SEQ, D_MODEL))
    Wqkv = np.asarray(Wqkv, dtype=np.float32)
    bqkv = np.asarray(bqkv, dtype=np.float32)
    Wo = np.asarray(Wo, dtype=np.float32)
    wq = np.asarray(wq, dtype=np.float32)
    wk = np.asarray(wk, dtype=np.float32)

    xta = np.zeros((DM_AUG, SEQ), np.float32)
    xta[:D_MODEL] = x2.T
    xta[D_MODEL] = 1.0
    xtb = np.ascontiguousarray(xta.astype(ml_dtypes.bfloat16))

    wkc = np.ascontiguousarray((wq * wk).reshape(D_HEAD, 1))

    in_maps = []
    for c in range(N_CORES):
        rows, brows = [], []
        for part in range(3):
            for h in (HEADS_LOCAL * c, HEADS_LOCAL * c + 1):
                sl = slice(part * D_MODEL + h * D_HEAD, part * D_MODEL + (h + 1) * D_HEAD)
                rows.append(Wqkv[sl])
                brows.append(bqkv[sl])
        Wl = np.concatenate(rows, 0)          # [384, 1024] rows [q0|q1|k0|k1|v0|v1]
        bl = np.concatenate(brows, 0)         # [384]
        wqkvta = np.zeros((DM_AUG, 384), np.float32)
        wqkvta[:D_MODEL] = Wl.T
        wqkvta[D_MODEL] = bl
        wkvt = np.ascontiguousarray(
            wqkvta[:, Q_LOCAL:].astype(ml_dtypes.bfloat16)
        )                                                              # [1152, 256]
        wqt = np.ascontiguousarray(
            wqkvta[:, :Q_LOCAL].astype(ml_dtypes.bfloat16)
        )                                                              # [1152, 128]
        cols = slice(HEADS_LOCAL * c * D_HEAD, (HEADS_LOCAL * c + HEADS_LOCAL) * D_HEAD)
        wotc = np.ascontiguousarray(Wo[:, cols].T.astype(ml_dtypes.bfloat16))
        in_maps.append(
            {
                "xtb": xtb,
                "wkvt": wkvt,
                "wqt": wqt,
                "wot": wotc,
                "wkc": wkc,
            }
        )
    return in_maps


_NC_CACHE = {}
LAST_RESULT = None


def kernel(x, Wqkv, bqkv, Wo, bo, wq, wk):
    import os
    from concourse.bass_utils import run_bass_kernel_spmd

    global LAST_RESULT
    assert np.asarray(x).shape == (1, SEQ, D_MODEL)
    in_maps = shard_inputs(x, Wqkv, bqkv, Wo, bo, wq, wk)
    if "nc" not in _NC_CACHE:
        _NC_CACHE["nc"] = build_core_kernel()
    nc = _NC_CACHE["nc"]
    trace = bool(int(os.environ.get("BASS_KERNEL_TRACE", "0")))
    res = run_bass_kernel_spmd(nc, in_maps, list(range(N_CORES)), trace=trace)
    LAST_RESULT = res
    acc = np.zeros((SEQ, D_MODEL), np.float64)
    for c in range(N_CORES):
        acc += res.results[c]["out"].astype(np.float64)
    acc += np.asarray(bo, dtype=np.float64)
    return acc.astype(np.float32).reshape(1, SEQ, D_MODEL)


# revision 44
# speedup vs baseline: 1.0068x; 1.0068x over previous
"""Trainium2 Bass kernel for a 16-head attention block (d_model=1024, seq=4096).

Sharding: tensor-parallel over heads. Each of the 8 cores computes QKV
projections, RMSNorm(q,k), full softmax(QK^T)V attention for its 2 heads,
and a partial O-projection (its heads' slice of the contraction). The host
sums the 8 partial outputs (bf16 partials) and adds the output bias.

Per-core dataflow (k-first, attention is ACT/exp-bound so everything else
is arranged to hide under it). x ships ONCE as bf16 and stays resident in
SBUF (8 chunk tiles, 1KB descriptor runs) — the cost model serializes all
DMA on a shared 360GB/s device, so halving x traffic halves the prefix:
  phase 1 (serial prefix): k,v projections (bf16 GEMM, moving dim 256),
           RMSNorm(k) with wq*wk folded into the k side, PE-transpose
           k_hat into [64, s] fp32r tiles, V -> bf16 [k, 65] chunks with a
           fused ones column. The q side for the first q-tile is
           interleaved into the prefix tail (ACT Sqrt path).
  phase 2 (ACT-bound steady state): per (q-tile 512, head):
           S[k,q] blocks via khatT.T @ qhatT (K=64 fp32r), exp on ACT ->
           bf16 probs, z[q,65] += probs_chunk.T @ V' with probs as the
           stationary operand (65-column moving operand halves PE time).
           Row 64 = softmax denominator; normalize in q-major on DVE,
           PE-transpose both heads at once into z_nT[d_local, s] bf16.
           The NEXT q-tile's q side runs under the exp shadow: bf16
           x @ Wq GEMM (N=128 bf16 runs at 1 cyc/row; fp32r would be 4x),
           RMSNorm(q) with a Newton-iteration rsqrt on DVE (keeps the
           ACT table on Exp), PE-transposes into qhatT.
  phase 3: out[s,dm] partial = z_nT.T @ WoT (bf16), PSUM->SBUF bf16 -> HBM,
           software-pipelined one q-tile behind attention.
"""

import numpy as np
from contextlib import ExitStack

import concourse.bass as bass
import concourse.tile as tile
from concourse import mybir
from concourse.masks import make_identity

F32 = mybir.dt.float32
F32R = mybir.dt.float32r
BF16 = mybir.dt.bfloat16
I32 = mybir.dt.int32
AF = mybir.ActivationFunctionType
ALU = mybir.AluOpType

D_MODEL = 1024
SEQ = 4096
N_HEADS = 16
D_HEAD = 64
N_CORES = 8
HEADS_LOCAL = 2
P = 128
DM_AUG = D_MODEL + P                     # 1152 rows: x^T plus ones-row block
NCH = DM_AUG // P                        # 9 contraction chunks
KV_LOCAL = 2 * HEADS_LOCAL * D_HEAD      # 256: [k0|k1|v0|v1]
Q_LOCAL = HEADS_LOCAL * D_HEAD           # 128: [q0|q1]
SB = SEQ // P                            # 32 s-blocks
QT = 8                                   # q-tiles of 512
QW = SEQ // QT                           # 512
QC = QW // P                             # 4 q-chunks of 128 per q-tile
KB = SEQ // P                            # 32 k-blocks
EXP_BATCH = 3
EPS = 1e-6
RSQRT_MAGIC = 0x5F3759DF


MAX_WAITS = 1


def _split_excess_waits(nc):
    """This walrus build rejects instructions carrying more than one or two
    sync-wait commands (CTRL and pseudo-DMA structs especially). Rewrite every
    instruction with more than MAX_WAITS waits into a chain of same-engine
    NoOps each carrying MAX_WAITS waits, followed by the original."""
    import bass_rust

    n_new = 0
    for f in nc.m.functions:
        for bb in f.blocks:
            changed = False
            out = []
            for ins in bb.instructions:
                si = ins.sync_info
                waits = list(si.on_wait) if si is not None and si.on_wait else []
                if len(waits) > MAX_WAITS:
                    changed = True
                    ncar = len(waits) - MAX_WAITS
                    for i in range(0, ncar, MAX_WAITS):
                        chunk = waits[i : min(i + MAX_WAITS, ncar)]
                        nop = mybir.InstNoOp(
                            name=f"{ins.name}-wsplit{i}", ins=[], outs=[]
                        )
                        nop.engine = ins.engine
                        nop.sync_info = bass_rust.SyncInfo(
                            on_wait=chunk, on_update=[]
                        )
                        out.append(nop)
                        n_new += 1
                    ins.sync_info = bass_rust.SyncInfo(
                        on_wait=waits[ncar:], on_update=si.on_update
                    )
                out.append(ins)
            if changed:
                bb.instructions = out
    return n_new


def build_core_kernel(split_waits=True):
    nc = bass.Bass()
    xtb = nc.declare_dram_parameter("xtb", [DM_AUG, SEQ], BF16, isOutput=False)
    wkvt = nc.declare_dram_parameter("wkvt", [DM_AUG, KV_LOCAL], BF16, isOutput=False)
    wqt = nc.declare_dram_parameter("wqt", [DM_AUG, Q_LOCAL], BF16, isOutput=False)
    wot = nc.declare_dram_parameter("wot", [P, D_MODEL], BF16, isOutput=False)
    wkc = nc.declare_dram_parameter("wkc", [D_HEAD, 1], F32, isOutput=False)
    out = nc.declare_dram_parameter("out", [SEQ, D_MODEL], BF16, isOutput=True)

    xtb_r = xtb.rearrange("(c p) s -> p c s", p=P)       # [128, 9, 4096]
    wkvt_r = wkvt.rearrange("(c p) f -> p c f", p=P)     # [128, 9, 256]
    wqt_r = wqt.rearrange("(c p) f -> p c f", p=P)       # [128, 9, 128]

    with ExitStack() as ctx:
        tc = ctx.enter_context(tile.TileContext(nc))

        const = ctx.enter_context(tc.tile_pool(name="const", bufs=1))
        persist = ctx.enter_context(tc.tile_pool(name="persist", bufs=1))

        # DMA order matters: the whole prefix waits on x chunk 0 + Wkv
        xb0 = const.tile([P, NCH, QW], BF16, name="xb0")
        wkv_sb = const.tile([P, NCH, KV_LOCAL], BF16)
        nc.sync.dma_start(wkv_sb[:, 0:3, :], wkvt_r[:, 0:3, :])
        nc.sync.dma_start(xb0[:, :, 0:QW // 2], xtb_r[:, :, 0 : QW // 2])
        nc.sync.dma_start(wkv_sb[:, 3:NCH, :], wkvt_r[:, 3:NCH, :])
        nc.sync.dma_start(xb0[:, :, QW // 2 : QW], xtb_r[:, :, QW // 2 : QW])
        ident_f = const.tile([P, P], F32)
        make_identity(nc, ident_f)
        ident_r = const.tile([P, P], F32R)
        nc.scalar.activation(ident_r[:], ident_f[:], AF.Copy)
        ident_b = const.tile([P, P], BF16)
        nc.vector.tensor_copy(ident_b[:], ident_f[:])
        wkc_sb = const.tile([D_HEAD, 1], F32)
        nc.sync.dma_start(wkc_sb[:], wkc[:])
        eps_t = const.tile([P, 1], F32)
        nc.gpsimd.memset(eps_t[:], EPS)
        wq_sb = const.tile([P, NCH, Q_LOCAL], BF16)
        nc.sync.dma_start(wq_sb[:], wqt_r)
        wot_sb = const.tile([P, D_MODEL], BF16)
        nc.sync.dma_start(wot_sb[:], wot[:])

        # attention operands: q_hat/k_hat in [64, s] fp32r (K=64 contraction),
        # V' in [k, 65] bf16 per (head, k-block) with ones denominator column
        qhatT = [persist.tile([D_HEAD, SEQ], F32R, name=f"qhatT{h}") for h in range(2)]
        khatT = [persist.tile([D_HEAD, SEQ], F32R, name=f"khatT{h}") for h in range(2)]
        vp = persist.tile([P, HEADS_LOCAL, KB, D_HEAD + 1], BF16)
        nc.gpsimd.memset(vp[:, :, :, D_HEAD : D_HEAD + 1], 1.0)
        z_nT = persist.tile([P, SEQ], BF16)

        # resident bf16 copy of x^T (augmented): 8 chunk tiles of 512 s-cols,
        # loaded once (1KB descriptor runs; serves both kv- and q-GEMMs)
        xball = [xb0] + [
            persist.tile([P, NCH, QW], BF16, name=f"xb{d}") for d in range(1, QT)
        ]
        for d in range(1, QT):
            nc.sync.dma_start(xball[d][:], xtb_r[:, :, bass.ts(d, QW)])

        qnorm = ctx.enter_context(tc.tile_pool(name="qnorm", bufs=3))

        def emit_q_side_sb(sb, qpool, tpool, rsqrt_act=False):
            """bf16 x @ Wq for one s-block + RMSNorm(q) + PE transposes into
            qhatT. rsqrt_act: use ACT Rsqrt (phase-1 table) instead of the
            DVE Newton rsqrt (phase 2 keeps the ACT table on Exp)."""
            if True:
                ssl = bass.ts(sb, P)
                xbt = xball[sb // QC]
                xsl = bass.ts(sb % QC, P)
                qps = qpool.tile([P, Q_LOCAL], F32, name="qps", tag=qpool.name_tag)
                for c in range(NCH):
                    nc.tensor.matmul(
                        qps[:],
                        lhsT=xbt[:, c, xsl],
                        rhs=wq_sb[:, c, :],
                        start=(c == 0),
                        stop=(c == NCH - 1),
                    )
                qsb = qnorm.tile([P, Q_LOCAL], F32, name="qsb", tag="qsb")
                nc.vector.tensor_copy(qsb[:], qps[:])
                qg = qsb[:].rearrange("p (g d) -> p g d", g=2)
                sq = qnorm.tile([P, 2, D_HEAD], F32, name="qsq", tag="qsq")
                nc.vector.tensor_tensor(sq[:], qg, qg, ALU.mult)
                ss = qnorm.tile([P, 2], F32, name="qss", tag="qss")
                nc.vector.tensor_reduce(
                    ss[:], sq[:], axis=mybir.AxisListType.X, op=ALU.add
                )
                # rr = rsqrt(ss/64 + eps) via bit-trick seed + 2 Newton steps
                if rsqrt_act:
                    yrs = qnorm.tile([P, 2], F32, name="qrs", tag="qrs")
                    nc.scalar.activation(
                        yrs[:], ss[:], AF.Sqrt, bias=eps_t[:], scale=1.0 / D_HEAD
                    )
                    yact = qnorm.tile([P, 2], F32, name="qra", tag="qra")
                    nc.vector.reciprocal(yact[:], yrs[:])
                    y = yact[:]
                else:
                    ms = qnorm.tile([P, 2], F32, name="qms", tag="qms")
                    nc.vector.tensor_scalar(
                        ms[:], ss[:], 1.0 / D_HEAD, EPS, op0=ALU.mult, op1=ALU.add
                    )
                    xh = qnorm.tile([P, 2], F32, name="qxh", tag="qxh")
                    nc.vector.tensor_scalar(xh[:], ms[:], 0.5, None, op0=ALU.mult)
                    iy = qnorm.tile([P, 2], I32, name="qiy", tag="qiy")
                    nc.vector.tensor_scalar(
                        iy[:], ms[:].bitcast(I32), 1, None, op0=ALU.logical_shift_right
                    )
                    nc.vector.tensor_scalar(
                        iy[:], iy[:], -1, RSQRT_MAGIC, op0=ALU.mult, op1=ALU.add
                    )
                    y = iy[:].bitcast(F32)
                    for it in range(2):
                        y2 = qnorm.tile([P, 2], F32, name=f"qy2_{it}", tag=f"qy2_{it}")
                        nc.vector.tensor_tensor(y2[:], y, y, ALU.mult)
                        nc.vector.tensor_tensor(y2[:], y2[:], xh[:], ALU.mult)
                        nc.vector.tensor_scalar(
                            y2[:], y2[:], -1.0, 1.5, op0=ALU.mult, op1=ALU.add
                        )
                        yn = qnorm.tile([P, 2], F32, name=f"qyn_{it}", tag=f"qyn_{it}")
                        nc.vector.tensor_tensor(yn[:], y, y2[:], ALU.mult)
                        y = yn[:]
                q_hat = qnorm.tile([P, 2, D_HEAD], F32R, name="qhat", tag="qhat")
                nc.vector.tensor_tensor(
                    q_hat[:], qg, y[:, :, None].to_broadcast((P, 2, D_HEAD)), ALU.mult
                )
                for h in range(2):
                    pt = tpool.tile(
                        [D_HEAD, P], F32R, name="qpt", tag=tpool.name_tag
                    )
                    nc.tensor.transpose(pt[:], q_hat[:, h, :], ident_r[:])
                    nc.vector.tensor_copy(qhatT[h][:, ssl], pt[:])

        # ------------- phase 1: K/V projections + RMSNorm(k) + V' -------------
        with ExitStack() as p1:
            norm = p1.enter_context(tc.tile_pool(name="norm", bufs=8))
            qkps = p1.enter_context(tc.tile_pool(name="kvps", bufs=6, space="PSUM"))
            tps = p1.enter_context(tc.tile_pool(name="tps", bufs=2, space="PSUM"))

            class _P1Pool:
                name_tag = "kvps"

                @staticmethod
                def tile(shape, dt, name=None, tag=None):
                    return qkps.tile(shape, dt, name=name, tag="kvps")

            class _P1TPool:
                name_tag = "tps"

                @staticmethod
                def tile(shape, dt, name=None, tag=None):
                    return tps.tile(shape, dt, name=name, tag="tps")

            for sb in range(SB):
                ssl = bass.ts(sb, P)
                xbt = xball[sb // QC]
                xsl = bass.ts(sb % QC, P)

                kv_ps = qkps.tile([P, KV_LOCAL], F32, name="kv_ps", tag="kvps")
                for c in range(NCH):
                    nc.tensor.matmul(
                        kv_ps[:],
                        lhsT=xbt[:, c, xsl],
                        rhs=wkv_sb[:, c, :],
                        start=(c == 0),
                        stop=(c == NCH - 1),
                    )

                # RMSNorm stats for the 2 k heads
                k_ps = kv_ps[:, 0 : 2 * D_HEAD].rearrange("p (g d) -> p g d", g=2)
                sq = norm.tile([P, 2, D_HEAD], F32)
                nc.scalar.activation(sq[:], k_ps, AF.Square)
                ss = norm.tile([P, 2], F32)
                nc.vector.tensor_reduce(
                    ss[:], sq[:], axis=mybir.AxisListType.X, op=ALU.add
                )
                rs = norm.tile([P, 2], F32)
                nc.scalar.activation(
                    rs[:], ss[:], AF.Sqrt, bias=eps_t[:], scale=1.0 / D_HEAD
                )
                rr = norm.tile([P, 2], F32)
                nc.vector.reciprocal(rr[:], rs[:])

                k_hat = norm.tile([P, 2, D_HEAD], F32R)
                nc.vector.tensor_tensor(
                    k_hat[:],
                    k_ps,
                    rr[:, :, None].to_broadcast((P, 2, D_HEAD)),
                    ALU.mult,
                )

                # V chunks for both heads -> bf16 (ACT: same table as Square)
                nc.scalar.activation(
                    vp[:, :, sb, 0:D_HEAD],
                    kv_ps[:, 2 * D_HEAD : 4 * D_HEAD].rearrange(
                        "p (h d) -> p h d", h=2
                    ),
                    AF.Copy,
                )

                # transposes into [d, s]; wq*wk folded into the k side
                for h in range(2):
                    pt = tps.tile([D_HEAD, P], F32R, name="pt", tag="tps")
                    nc.tensor.transpose(pt[:], k_hat[:, h, :], ident_r[:])
                    if h == 0:
                        nc.scalar.activation(
                            khatT[h][:, ssl], pt[:], AF.Copy, scale=wkc_sb[:]
                        )
                    else:
                        nc.vector.tensor_scalar_mul(
                            khatT[h][:, ssl], pt[:], wkc_sb[:]
                        )

                # q side of the first q-tile, interleaved into the prefix
                # tail (ACT Sqrt: phase 1 owns the sqrt table)
                if sb >= 17 and (sb - 17) % 4 == 0:
                    emit_q_side_sb((sb - 17) // 4, _P1Pool, _P1TPool,
                                   rsqrt_act=True)




        # ---------- phase 2+3: attention with inlined O-projection ----------
        # PSUM banks: 2 score slots x3 banks, z accumulator 1 bank, shared
        # utility bank ("ops": O-proj / next-q-tile GEMM+transposes / ztp) = 8
        with ExitStack() as p2:
            spool = p2.enter_context(tc.tile_pool(name="sps", bufs=2, space="PSUM"))
            zqpool = p2.enter_context(tc.tile_pool(name="zqps", bufs=1, space="PSUM"))
            opool = p2.enter_context(tc.tile_pool(name="ops", bufs=1, space="PSUM"))
            ppool = p2.enter_context(tc.tile_pool(name="probs", bufs=4))
            znpool = p2.enter_context(tc.tile_pool(name="zn", bufs=3))
            rpool = p2.enter_context(tc.tile_pool(name="rcp", bufs=3))
            osb = p2.enter_context(tc.tile_pool(name="osb", bufs=4))

            class _P2Pool:
                name_tag = "ops"

                @staticmethod
                def tile(shape, dt, name=None, tag=None):
                    return opool.tile(shape, dt, name=name, tag="ops")

            def emit_oproj(qt, final=False):
                # final q-tile: S slots are free, so pipeline the matmuls
                # 2-wide through them and put half the copies on the idle ACT
                for sbl in range(QC):
                    sb = qt * QC + sbl
                    ot = osb.tile([P, D_MODEL], BF16, name="ot", tag="ot")
                    for half in range(2):
                        if final:
                            ops = spool.tile([P, QW], F32, name="ops", tag="sps")
                        else:
                            ops = opool.tile([P, QW], F32, name="ops", tag="ops")
                        nc.tensor.matmul(
                            ops[:],
                            lhsT=z_nT[:, bass.ts(sb, P)],
                            rhs=wot_sb[:, bass.ts(half, QW)],
                            start=True,
                            stop=True,
                        )
                        if final and half == 0:
                            nc.scalar.activation(
                                ot[:, bass.ts(half, QW)], ops[:], AF.Copy
                            )
                        else:
                            nc.vector.tensor_copy(ot[:, bass.ts(half, QW)], ops[:])
                        if final:
                            nc.sync.dma_start(
                                out[bass.ts(sb, P), bass.ts(half, QW)],
                                ot[:, bass.ts(half, QW)],
                            )
                    if not final:
                        nc.sync.dma_start(out[bass.ts(sb, P), :], ot[:])

            for qt in range(QT):
                qsl = bass.ts(qt, QW)
                zn = znpool.tile([P, QC, P], BF16, name="zn", tag="zn")
                for h in range(HEADS_LOCAL):
                    zq = zqpool.tile([P, QC, D_HEAD + 1], F32, name="zq", tag="zq")
                    for kb0 in [0] + list(range(2, KB, EXP_BATCH)):
                        nb = 2 if kb0 == 0 else min(EXP_BATCH, KB - kb0)
                        sps = spool.tile(
                            [P, EXP_BATCH, QW], F32, name="sps", tag="sps"
                        )
                        for j in range(nb):
                            kb = kb0 + j
                            nc.tensor.matmul(
                                sps[:, j, :],
                                lhsT=khatT[h][:, bass.ts(kb, P)],
                                rhs=qhatT[h][:, qsl],
                                start=True,
                                stop=True,
                            )
                        probs = ppool.tile(
                            [P, EXP_BATCH, QW], BF16, name="probs", tag="probs"
                        )
                        nc.scalar.activation(
                            probs[:, 0:nb, :], sps[:, 0:nb, :], AF.Exp
                        )
                        # all 128 PV matmuls form ONE PSUM accumulation group
                        # (zq spans a single 2KB zero region): start marks the
                        # whole region pending-zero, each chunk's first touch
                        # overwrites, everything else accumulates
                        for j in range(nb):
                            kb = kb0 + j
                            for qc in range(QC):
                                nc.tensor.matmul(
                                    zq[:, qc, :],
                                    lhsT=probs[:, j, bass.ts(qc, P)],
                                    rhs=vp[:, h, kb, :],
                                    start=(kb == 0 and qc == 0),
                                    stop=(kb == KB - 1 and qc == QC - 1),
                                    skip_group_check=True,
                                )
                        # software-pipelined work emitted under the exp shadow:
                        # h0: O-projection of the previous q-tile
                        # h1: q side (GEMM+norm+transposes) of the next q-tile
                        if kb0 == 0 and h == 0 and qt > 0:
                            emit_oproj(qt - 1)
                        if kb0 == 0 and h == 1 and qt < QT - 1:
                            for sbl in range(QC):
                                emit_q_side_sb((qt + 1) * QC + sbl, _P2Pool, _P2Pool)
                    # normalize in q-major: z = z / rowsum (col 64)
                    rcp = rpool.tile([P, QC], F32, name="rcp", tag="rcp")
                    nc.vector.reciprocal(rcp[:], zq[:, :, D_HEAD])
                    nc.vector.tensor_tensor(
                        zn[:, :, bass.ts(h, D_HEAD)],
                        zq[:, :, 0:D_HEAD],
                        rcp[:, :, None].to_broadcast((P, QC, D_HEAD)),
                        ALU.mult,
                    )
                # transpose both heads at once into z_nT[d_local, s]
                ztp = opool.tile([P, QC, P], BF16, name="ztp", tag="ops")
                for qc in range(QC):
                    nc.tensor.transpose(ztp[:, qc, :], zn[:, qc, :], ident_b[:])
                    nc.vector.tensor_copy(
                        z_nT[:, qt * QW + qc * P : qt * QW + (qc + 1) * P],
                        ztp[:, qc, :],
                    )
            emit_oproj(QT - 1, final=True)

    if split_waits:
        _split_excess_waits(nc)
    return nc


def shard_inputs(x, Wqkv, bqkv, Wo, bo, wq, wk):
    import ml_dtypes

    x2 = np.ascontiguousarray(np.asarray(x, dtype=np.float32).reshape(SEQ, D_MODEL))
    Wqkv = np.asarray(Wqkv, dtype=np.float32)
    bqkv = np.asarray(bqkv, dtype=np.float32)
    Wo = np.asarray(Wo, dtype=np.float32)
    wq = np.asarray(wq, dtype=np.float32)
    wk = np.asarray(wk, dtype=np.float32)

    xta = np.zeros((DM_AUG, SEQ), np.float32)
    xta[:D_MODEL] = x2.T
    xta[D_MODEL] = 1.0
    xtb = np.ascontiguousarray(xta.astype(ml_dtypes.bfloat16))

    wkc = np.ascontiguousarray((wq * wk).reshape(D_HEAD, 1))

    in_maps = []
    for c in range(N_CORES):
        rows, brows = [], []
        for part in range(3):
            for h in (HEADS_LOCAL * c, HEADS_LOCAL * c + 1):
                sl = slice(part * D_MODEL + h * D_HEAD, part * D_MODEL + (h + 1) * D_HEAD)
                rows.append(Wqkv[sl])
                brows.append(bqkv[sl])
        Wl = np.concatenate(rows, 0)          # [384, 1024] rows [q0|q1|k0|k1|v0|v1]
        bl = np.concatenate(brows, 0)         # [384]
        wqkvta = np.zeros((DM_AUG, 384), np.float32)
        wqkvta[:D_MODEL] = Wl.T
        wqkvta[D_MODEL] = bl
        wkvt = np.ascontiguousarray(
            wqkvta[:, Q_LOCAL:].astype(ml_dtypes.bfloat16)
        )                                                              # [1152, 256]
        wqt = np.ascontiguousarray(
            wqkvta[:, :Q_LOCAL].astype(ml_dtypes.bfloat16)
        )                                                              # [1152, 128]
        cols = slice(HEADS_LOCAL * c * D_HEAD, (HEADS_LOCAL * c + HEADS_LOCAL) * D_HEAD)
        wotc = np.ascontiguousarray(Wo[:, cols].T.astype(ml_dtypes.bfloat16))
        in_maps.append(
            {
                "xtb": xtb,
                "wkvt": wkvt,
                "wqt": wqt,
                "wot": wotc,
                "wkc": wkc,
            }
        )
    return in_maps


_NC_CACHE = {}
LAST_RESULT = None


def kernel(x, Wqkv, bqkv, Wo, bo, wq, wk):
    import os
    from concourse.bass_utils import run_bass_kernel_spmd

    global LAST_RESULT
    assert np.asarray(x).shape == (1, SEQ, D_MODEL)
    in_maps = shard_inputs(x, Wqkv, bqkv, Wo, bo, wq, wk)
    if "nc" not in _NC_CACHE:
        _NC_CACHE["nc"] = build_core_kernel()
    nc = _NC_CACHE["nc"]
    trace = bool(int(os.environ.get("BASS_KERNEL_TRACE", "0")))
    res = run_bass_kernel_spmd(nc, in_maps, list(range(N_CORES)), trace=trace)
    LAST_RESULT = res
    acc = np.zeros((SEQ, D_MODEL), np.float64)
    for c in range(N_CORES):
        acc += res.results[c]["out"].astype(np.float64)
    acc += np.asarray(bo, dtype=np.float64)
    return acc.astype(np.float32).reshape(1, SEQ, D_MODEL)
